# revision 6
# baseline (speedup 1.0000x reference)
"""2-layer weighted-GCN embedding kernel for 8 Trainium2 NeuronCores.

Strategy (dst-sharded message passing, transposed dataflow):
  - Nodes are sharded by destination across the 8 cores (12500 each, padded
    to 12544 = 98 * 128).  Each core handles every edge whose dst lands in
    its shard, so the scatter-add is purely local.
  - GCN associativity: conv(x) = (A_hat @ x) @ W^T + b, so we aggregate RAW
    features first and apply the dense transform on the (sharded) aggregate.
  - Self-loops are regular edges; the full symmetric normalization
    dinv[src] * w * dinv[dst] is folded into the host-built indicator.
  - Scatter-add is an indicator matmul with the message block STATIONARY:
    acc[f, dst] = sum_slots msg[slot, f] * ind[slot, dst] comes out
    feature-partitioned, so the dense W matmul consumes it directly
    (lhsT = W^T) with no transpose in between.
  - Layer-0 messages x[src[slot]] are a compile-time permutation of the
    input, pre-gathered on the host and streamed sequentially (HWDGE).
    Layer-1 messages are gathered per-edge from the AllGather'd hidden
    state with SWDGE dma_gather, spread across all 4 SWDGE queues for
    4x parallel descriptor emission.
  - Between the two conv layers one AllGather shares r1 = relu(conv1)
    across cores (bf16).

kernel(**inputs) takes the FULL inputs and returns the FULL [100000, 64]
output; everything (sharding, compile, SPMD run, gather of shards) happens
inside.
"""

import numpy as np
import ml_dtypes

import concourse.bass as bass
import concourse.tile as tile
import concourse.bacc as bacc
from concourse import mybir, bass_utils

BF16 = ml_dtypes.bfloat16

F = 128
HID = 128
ENC = 64
NCORES = 8
SUBW = 128
SUPSZ = 4                      # subtiles per supertile (one gather covers these)


def _set_dims(n):
    """(Re)compute the node-count-derived global dims."""
    global N, SHARD, NSUB, SHARD_PAD, CHUNK, XROWS, NSUP
    N = n
    SHARD = N // NCORES
    NSUB = -(-SHARD // SUBW)           # subtiles per shard
    SHARD_PAD = NSUB * SUBW
    CHUNK = 2 * SHARD_PAD              # rows per gather chunk (< 2**15)
    XROWS = NCORES * SHARD_PAD         # padded node-table rows
    NSUP = -(-NSUB // SUPSZ)


NCHUNK = 4
_set_dims(100000)

_cache = {}


def _preprocess(x, edge_index, edge_weight, W1, b1, W2, b2, Wf, bf):
    """All host-side numpy prep: normalization, edge partitioning, layouts."""
    src = np.asarray(edge_index[0], dtype=np.int64)
    dst = np.asarray(edge_index[1], dtype=np.int64)
    w = np.asarray(edge_weight, dtype=np.float32)
    x = np.asarray(x, dtype=np.float32)

    deg = np.bincount(dst, weights=w.astype(np.float64), minlength=N) + 1.0
    dinv = (1.0 / np.sqrt(deg)).astype(np.float32)

    # self-loops as regular edges, full norm on every edge
    loop = np.arange(N, dtype=np.int64)
    src_f = np.concatenate([src, loop])
    dst_f = np.concatenate([dst, loop])
    nrm_f = np.concatenate([dinv[src] * w * dinv[dst], dinv * dinv])

    x_pad = np.zeros((XROWS, F), np.float32)
    for o in range(NCORES):
        x_pad[o * SHARD_PAD:o * SHARD_PAD + SHARD] = x[o * SHARD:(o + 1) * SHARD]
    x_bf = x_pad.astype(BF16)

    # map src node id -> (chunk, local row) in the padded table
    owner = src_f // SHARD
    src_pad = owner * SHARD_PAD + (src_f - owner * SHARD)
    chunk = src_pad // CHUNK
    src_local = (src_pad - chunk * CHUNK).astype(np.int64)
    assert src_local.max() < 2 ** 15

    NCELL = NCHUNK * NSUB  # flat cell id = c * NSUB + t

    # per-device cell contents
    dev = []
    counts = np.zeros((NCORES, NCELL), np.int64)
    for d in range(NCORES):
        lo, hi = d * SHARD, (d + 1) * SHARD
        m = (dst_f >= lo) & (dst_f < hi)
        dl = dst_f[m] - lo
        t = dl // SUBW
        cid = chunk[m] * NSUB + t
        order = np.argsort(cid, kind="stable")
        cid_s = cid[order]
        counts[d] = np.bincount(cid_s, minlength=NCELL)
        dev.append((cid_s,
                    src_local[m][order].astype(np.int16),
                    (dl % SUBW)[order].astype(np.int64),
                    nrm_f[m][order]))

    nb_cell = -(-counts.max(axis=0) // 128)            # blocks per cell (shared)
    cell_off = np.zeros(NCELL + 1, np.int64)
    np.cumsum(nb_cell * 128, out=cell_off[1:])
    TOT = int(cell_off[-1])

    per_core = []
    for d in range(NCORES):
        cid_s, sl, dr, nr = dev[d]
        starts = np.zeros(NCELL + 1, np.int64)
        np.cumsum(counts[d], out=starts[1:])
        rank = np.arange(len(cid_s)) - starts[cid_s]
        pos = cell_off[cid_s] + rank
        f_src = np.zeros(TOT, np.int16)
        f_dst = np.zeros(TOT, np.int64)
        f_nrm = np.zeros(TOT, np.float32)
        f_src[pos] = sl
        f_dst[pos] = dr
        f_nrm[pos] = nr
        f_chunk = np.zeros(TOT, np.int64)
        f_chunk[pos] = cid_s // NSUB

        idx16 = np.ascontiguousarray(np.tile(f_src.reshape(-1, 16).T, (8, 1)))

        # host-built indicators, partition-major:
        # indb[slot%128, (slot//128)*128 + dst_rel] = norm
        indb = np.zeros((128, TOT), BF16)
        posa = np.arange(TOT)
        indb[posa % 128, (posa // 128) * 128 + f_dst] = f_nrm.astype(BF16)

        # host pre-gathered layer-0 messages, same [128, TOT] layout as the
        # SBUF msg tiles: msg0[slot%128, (slot//128)*128 + f] = x[src[slot], f]
        gl = f_chunk * CHUNK + f_src.astype(np.int64)
        rows = x_bf[gl]                                  # [TOT, F]
        msg0 = np.ascontiguousarray(
            rows.reshape(TOT // 128, 128, F).transpose(1, 0, 2).reshape(128, TOT))

        per_core.append({
            "idx16": idx16,
            "indb": indb,
            "msg0": msg0,
        })

    shared = {
        "w1t": np.ascontiguousarray(np.asarray(W1, np.float32).T.astype(BF16)),
        "w2t": np.ascontiguousarray(np.asarray(W2, np.float32).T.astype(BF16)),
        "wft": np.ascontiguousarray(np.asarray(Wf, np.float32).T.astype(BF16)),
        "b1col": np.asarray(b1, np.float32).reshape(128, 1).copy(),
        "b2col": np.asarray(b2, np.float32).reshape(128, 1).copy(),
        "bfbc": np.broadcast_to(np.asarray(bf, np.float32), (128, ENC)).copy(),
        "identb": np.eye(128, dtype=np.float32).astype(BF16),
    }
    nb = nb_cell.reshape(NCHUNK, NSUB)      # [c][t]
    offs = cell_off.reshape(-1)             # flat slot offsets, id = c*NSUB+t
    return shared, per_core, nb, offs, TOT


def _build(nb, offs, TOT):
    """Build the SPMD bass program (identical for all 8 cores)."""
    nc = bacc.Bacc("TRN2", target_bir_lowering=False, debug=False,
                   num_devices=NCORES, num_swdge_queues=4)
    f32 = mybir.dt.float32
    bf16 = mybir.dt.bfloat16

    idx16_t = nc.dram_tensor("idx16", [128, TOT // 16], mybir.dt.int16, kind="ExternalInput")
    indb_t = nc.dram_tensor("indb", [128, TOT], bf16, kind="ExternalInput")
    msg0_t = nc.dram_tensor("msg0", [128, TOT], bf16, kind="ExternalInput")
    w1t_t = nc.dram_tensor("w1t", [F, HID], bf16, kind="ExternalInput")
    w2t_t = nc.dram_tensor("w2t", [HID, HID], bf16, kind="ExternalInput")
    wft_t = nc.dram_tensor("wft", [HID, ENC], bf16, kind="ExternalInput")
    b1col_t = nc.dram_tensor("b1col", [128, 1], f32, kind="ExternalInput")
    b2col_t = nc.dram_tensor("b2col", [128, 1], f32, kind="ExternalInput")
    bfbc_t = nc.dram_tensor("bfbc", [128, ENC], f32, kind="ExternalInput")
    identb_t = nc.dram_tensor("identb", [128, 128], bf16, kind="ExternalInput")
    out_t = nc.dram_tensor("out", [SHARD_PAD, ENC], f32, kind="ExternalOutput")

    # per-subtile block lists: blocks[t] = ordered [(c, k), ...]
    blocks = [[(c, k) for c in range(NCHUNK) for k in range(int(nb[c][t]))]
              for t in range(NSUB)]

    with tile.TileContext(nc) as tc:
        with tc.tile_pool(name="const", bufs=1) as cst, \
             tc.tile_pool(name="edata", bufs=1) as edata, \
             tc.tile_pool(name="msgp", bufs=3) as msgp, \
             tc.tile_pool(name="indp", bufs=3) as indp, \
             tc.tile_pool(name="accp", bufs=3, space="PSUM") as accp, \
             tc.tile_pool(name="epsp", bufs=3, space="PSUM") as epsp, \
             tc.tile_pool(name="tpsp", bufs=2, space="PSUM") as tpsp, \
             tc.tile_pool(name="work", bufs=3) as work, \
             tc.tile_pool(name="dram", bufs=1, space="DRAM") as dram:

            # ---- persistent SBUF data ----
            idx_sb = edata.tile([128, TOT // 16], mybir.dt.int16)
            nc.sync.dma_start(idx_sb[:], idx16_t[:])

            w1t_sb = cst.tile([F, HID], bf16)
            w2t_sb = cst.tile([HID, HID], bf16)
            wft_sb = cst.tile([HID, ENC], bf16)
            b1col_sb = cst.tile([128, 1], f32)
            b2col_sb = cst.tile([128, 1], f32)
            bfbc_sb = cst.tile([128, ENC], f32)
            ident_sb = cst.tile([128, 128], bf16)
            for sb_, t_ in ((w1t_sb, w1t_t), (w2t_sb, w2t_t), (wft_sb, wft_t),
                            (b1col_sb, b1col_t), (b2col_sb, b2col_t),
                            (bfbc_sb, bfbc_t), (ident_sb, identb_t)):
                nc.sync.dma_start(sb_[:], t_[:])

            r1sh = dram.tile([SHARD_PAD, HID], bf16)
            r1full = dram.tile([XROWS, HID], bf16, addr_space="Shared")

            def issue_loads(layer, s):
                """Issue the msg + indicator loads for supertile s."""
                subs = list(range(s * SUPSZ, min((s + 1) * SUPSZ, NSUB)))
                msgs = {}
                inds = {}
                starts = {}
                for c in range(NCHUNK):
                    start_slot = int(offs[c * NSUB + subs[0]])
                    end_slot = int(offs[c * NSUB + subs[-1] + 1])
                    L = end_slot - start_slot
                    if L == 0:
                        continue
                    starts[c] = start_slot
                    msg = msgp.tile([128, L], bf16, tag=f"msg{c}")
                    msgs[c] = msg
                    if layer == 0:
                        nc.sync.dma_start(
                            msg[:], msg0_t[:, start_slot:end_slot])
                    else:
                        nc.gpsimd.dma_gather(
                            msg[:].rearrange("p (b f) -> p b f", f=128),
                            r1full[c * CHUNK:(c + 1) * CHUNK, :],
                            idx_sb[:, start_slot // 16:end_slot // 16],
                            L, L, 128, elem_step=F,
                            single_packet=False,
                            queue_num=c,
                        )
                    ind = indp.tile([128, L], bf16, tag=f"ind{c}")
                    inds[c] = ind
                    nc.scalar.dma_start(
                        ind[:], indb_t[:, start_slot:end_slot])
                return subs, msgs, inds, starts

            def aggregate_layer(layer):
                """Messages + indicator -> transposed aggregate -> dense.

                layer 0: messages streamed from host-pregathered msg0.
                layer 1: messages gathered per-edge from r1full.
                Software-pipelined: supertile s+1's loads are issued before
                supertile s's compute, so stores never head-of-line-block
                the next loads in the per-engine instruction streams."""
                staged = issue_loads(layer, 0)
                for s in range(NSUP):
                    subs, msgs, inds, starts = staged
                    if s + 1 < NSUP:
                        staged = issue_loads(layer, s + 1)

                    # ---- per-subtile accumulate + epilogue ----
                    for t in subs:
                        acc = accp.tile([128, 128], f32, tag="acc")
                        for c in range(NCHUNK):
                            nbk = int(nb[c][t])
                            if nbk == 0:
                                continue
                            base = int(offs[c * NSUB + t]) - starts[c]
                            for k in range(nbk):
                                o = base + k * 128
                                nc.tensor.matmul(
                                    acc[:],
                                    lhsT=msgs[c][:, o:o + 128],
                                    rhs=inds[c][:, o:o + 128],
                                    start=(blocks[t][0] == (c, k)),
                                    stop=(blocks[t][-1] == (c, k)),
                                )

                        # sum over slots now sits as acc[f, dst] in PSUM
                        sum_sb = work.tile([128, 128], bf16, tag="sum")
                        nc.vector.tensor_copy(out=sum_sb[:], in_=acc[:])

                        if layer == 0:
                            z1 = epsp.tile([128, 128], f32, tag="eps")
                            nc.tensor.matmul(z1[:], lhsT=w1t_sb[:], rhs=sum_sb[:],
                                             start=True, stop=True)
                            r1t = work.tile([128, 128], bf16, tag="r1t")
                            nc.scalar.activation(
                                r1t[:], z1[:],
                                mybir.ActivationFunctionType.Relu,
                                bias=b1col_sb[:, 0:1])
                            rp = tpsp.tile([128, 128], bf16, tag="tp")
                            nc.tensor.transpose(rp[:], r1t[:], ident_sb[:])
                            r1 = work.tile([128, HID], bf16, tag="r1")
                            nc.scalar.activation(
                                r1[:], rp[:],
                                mybir.ActivationFunctionType.Copy)
                            nc.sync.dma_start(
                                r1sh[t * 128:(t + 1) * 128, :], r1[:])
                        else:
                            z2 = epsp.tile([128, 128], f32, tag="eps")
                            nc.tensor.matmul(z2[:], lhsT=w2t_sb[:], rhs=sum_sb[:],
                                             start=True, stop=True)
                            r2t = work.tile([128, 128], bf16, tag="r1t")
                            nc.scalar.activation(
                                r2t[:], z2[:],
                                mybir.ActivationFunctionType.Relu,
                                bias=b2col_sb[:, 0:1])
                            fp = tpsp.tile([128, ENC], f32, tag="tp")
                            nc.tensor.matmul(fp[:], lhsT=r2t[:], rhs=wft_sb[:],
                                             start=True, stop=True)
                            fz = work.tile([128, ENC], f32, tag="fz")
                            nc.vector.tensor_tensor(out=fz[:], in0=fp[:],
                                                    in1=bfbc_sb[:],
                                                    op=mybir.AluOpType.add)
                            nc.sync.dma_start(
                                out_t[t * 128:(t + 1) * 128, :], fz[:])

            aggregate_layer(0)
            nc.gpsimd.collective_compute(
                "AllGather",
                mybir.AluOpType.bypass,
                replica_groups=[list(range(NCORES))],
                ins=[r1sh[:].opt()],
                outs=[r1full[:].opt()],
            )
            aggregate_layer(1)

    nc.compile()
    return nc


def kernel(**inputs):
    shared, per_core, nb, offs, TOT = _preprocess(
        inputs["x"], inputs["edge_index"], inputs["edge_weight"],
        inputs["W1"], inputs["b1"], inputs["W2"], inputs["b2"],
        inputs["Wf"], inputs["bf"])

    key = (TOT, nb.tobytes())
    if key not in _cache:
        _cache[key] = _build(nb, offs, TOT)
    nc = _cache[key]

    in_maps = []
    for d in range(NCORES):
        m = dict(shared)
        m.update(per_core[d])
        in_maps.append(m)

    res = bass_utils.run_bass_kernel_spmd(nc, in_maps, core_ids=list(range(NCORES)))
    out = np.concatenate(
        [res.results[d]["out"][:SHARD] for d in range(NCORES)], axis=0)
    return out.astype(np.float32)


# revision 10
# speedup vs baseline: 1.0715x; 1.0715x over previous
"""2-layer weighted-GCN embedding kernel for 8 Trainium2 NeuronCores.

Strategy (dst-sharded message passing, transposed dataflow):
  - Nodes are sharded by destination across the 8 cores (12500 each, padded
    to 12544 = 98 * 128).  Each core handles every edge whose dst lands in
    its shard, so the scatter-add is purely local.
  - GCN associativity: conv(x) = (A_hat @ x) @ W^T + b, so we aggregate RAW
    features first and apply the dense transform on the (sharded) aggregate.
  - Self-loops are regular edges; the full symmetric normalization
    dinv[src] * w * dinv[dst] is folded into the host-built indicator.
  - Scatter-add is an indicator matmul with the message block STATIONARY:
    acc[f, dst] = sum_slots msg[slot, f] * ind[slot, dst] comes out
    feature-partitioned, so the dense W matmul consumes it directly
    (lhsT = W^T) with no transpose in between.
  - Layer 0: messages x[src[slot]] are a compile-time permutation of the
    input, pre-gathered on the host, interleaved with the indicator blocks
    in one flat per-supertile-contiguous DRAM stream -> a single maximally
    efficient HWDGE load per supertile.  Cells are per-subtile (no chunk
    split), so padding is ~5%.
  - Layer 1: messages are gathered per-edge from the AllGather'd hidden
    state with SWDGE dma_gather (indices int16 -> the 100352-row table is
    split in 4 chunks of 25088 rows), spread across all 4 SWDGE queues for
    4x parallel descriptor emission, each chunk split in 2 gather calls so
    SDMA drain overlaps Q7 emission.  Indicators stream as one contiguous
    load per supertile.

kernel(**inputs) takes the FULL inputs and returns the FULL [100000, 64]
output; everything (sharding, compile, SPMD run, gather of shards) happens
inside.
"""

import numpy as np
import ml_dtypes

import concourse.bass as bass
import concourse.tile as tile
import concourse.bacc as bacc
from concourse import mybir, bass_utils

BF16 = ml_dtypes.bfloat16

F = 128
HID = 128
ENC = 64
NCORES = 8
SUBW = 128
SUPSZ = 3                      # subtiles per supertile
NCHUNK = 4                     # gather chunks (int16 index limit), layer 1 only


def _set_dims(n):
    global N, SHARD, NSUB, SHARD_PAD, CHUNK, XROWS, NSUP
    N = n
    SHARD = N // NCORES
    NSUB = -(-SHARD // SUBW)           # subtiles per shard
    SHARD_PAD = NSUB * SUBW
    CHUNK = 2 * SHARD_PAD              # rows per gather chunk (< 2**15)
    XROWS = NCORES * SHARD_PAD         # padded node-table rows
    NSUP = -(-NSUB // SUPSZ)


_set_dims(100000)

_cache = {}


def _preprocess(x, edge_index, edge_weight, W1, b1, W2, b2, Wf, bf):
    """All host-side numpy prep: normalization, edge partitioning, layouts."""
    src = np.asarray(edge_index[0], dtype=np.int64)
    dst = np.asarray(edge_index[1], dtype=np.int64)
    w = np.asarray(edge_weight, dtype=np.float32)
    x = np.asarray(x, dtype=np.float32)

    deg = np.bincount(dst, weights=w.astype(np.float64), minlength=N) + 1.0
    dinv = (1.0 / np.sqrt(deg)).astype(np.float32)

    # self-loops as regular edges, full norm on every edge
    loop = np.arange(N, dtype=np.int64)
    src_f = np.concatenate([src, loop])
    dst_f = np.concatenate([dst, loop])
    nrm_f = np.concatenate([dinv[src] * w * dinv[dst], dinv * dinv])

    x_pad = np.zeros((XROWS, F), np.float32)
    for o in range(NCORES):
        x_pad[o * SHARD_PAD:o * SHARD_PAD + SHARD] = x[o * SHARD:(o + 1) * SHARD]
    x_bf = x_pad.astype(BF16)

    # padded global row of each edge's src
    owner = src_f // SHARD
    src_pad = owner * SHARD_PAD + (src_f - owner * SHARD)
    chunk = src_pad // CHUNK
    src_local = (src_pad - chunk * CHUNK).astype(np.int64)
    assert src_local.max() < 2 ** 15

    core_of = dst_f // SHARD
    dst_l_all = dst_f - core_of * SHARD
    t_all = dst_l_all // SUBW
    dr_all = dst_l_all % SUBW

    # ---------- layer-0 layout: cells = subtile only ----------
    c0 = np.zeros((NCORES, NSUB), np.int64)
    for d in range(NCORES):
        c0[d] = np.bincount(t_all[core_of == d], minlength=NSUB)
    nb0 = -(-c0.max(axis=0) // 128)                     # blocks per subtile
    off0 = np.zeros(NSUB + 1, np.int64)
    np.cumsum(nb0 * 128, out=off0[1:])
    TOT0 = int(off0[-1])

    # ---------- layer-1 layout: cells = chunk x subtile ----------
    NCELL = NCHUNK * NSUB
    cid_all = chunk * NSUB + t_all
    c1 = np.zeros((NCORES, NCELL), np.int64)
    for d in range(NCORES):
        c1[d] = np.bincount(cid_all[core_of == d], minlength=NCELL)
    nb1 = -(-c1.max(axis=0) // 128)
    off1 = np.zeros(NCELL + 1, np.int64)
    np.cumsum(nb1 * 128, out=off1[1:])
    TOT1 = int(off1[-1])

    # per-supertile spans
    sup_subs = [list(range(s * SUPSZ, min((s + 1) * SUPSZ, NSUB)))
                for s in range(NSUP)]
    # layer-0 flat stream offsets (in columns, per supertile: msg | ind)
    mi0_off = []
    o = 0
    for subs in sup_subs:
        L = int(off0[subs[-1] + 1] - off0[subs[0]])
        mi0_off.append((o, L))
        o += 2 * L
    MI0_COLS = o
    # layer-1 flat ind stream offsets
    ind1_off = []
    o = 0
    for subs in sup_subs:
        spans = []
        for c in range(NCHUNK):
            st = int(off1[c * NSUB + subs[0]])
            en = int(off1[c * NSUB + subs[-1] + 1])
            spans.append((st, en - st))
        base = o
        o += sum(L for _, L in spans)
        ind1_off.append((base, spans))
    IND1_COLS = o

    per_core = []
    for d in range(NCORES):
        m = core_of == d
        t_d = t_all[m]
        dr_d = dr_all[m]
        nrm_d = nrm_f[m].astype(np.float32)
        gl_d = src_pad[m]
        sl_d = src_local[m]
        ch_d = chunk[m]

        # ----- layer 0: slots ordered by subtile cell -----
        order0 = np.argsort(t_d, kind="stable")
        starts = np.zeros(NSUB + 1, np.int64)
        np.cumsum(c0[d], out=starts[1:])
        rank = np.arange(len(t_d)) - starts[t_d[order0]]
        pos0 = off0[t_d[order0]] + rank
        g_src0 = np.zeros(TOT0, np.int64)
        f_dst0 = np.zeros(TOT0, np.int64)
        f_nrm0 = np.zeros(TOT0, np.float32)
        g_src0[pos0] = gl_d[order0]
        f_dst0[pos0] = dr_d[order0]
        f_nrm0[pos0] = nrm_d[order0]

        rows = x_bf[g_src0]                              # [TOT0, F]
        msg0 = rows.reshape(TOT0 // 128, 128, F).transpose(1, 0, 2).reshape(128, TOT0)
        ind0 = np.zeros((128, TOT0), BF16)
        posa = np.arange(TOT0)
        ind0[posa % 128, (posa // 128) * 128 + f_dst0] = f_nrm0.astype(BF16)

        mi0 = np.empty(128 * MI0_COLS, BF16)
        for s, subs in enumerate(sup_subs):
            base, L = mi0_off[s]
            st = int(off0[subs[0]])
            blk = np.concatenate([msg0[:, st:st + L], ind0[:, st:st + L]], axis=1)
            mi0[128 * base:128 * (base + 2 * L)] = blk.reshape(-1)

        # ----- layer 1: slots ordered by (chunk, subtile) cell -----
        cid_d = ch_d * NSUB + t_d
        order1 = np.argsort(cid_d, kind="stable")
        starts = np.zeros(NCELL + 1, np.int64)
        np.cumsum(c1[d], out=starts[1:])
        rank = np.arange(len(cid_d)) - starts[cid_d[order1]]
        pos1 = off1[cid_d[order1]] + rank
        f_src1 = np.zeros(TOT1, np.int16)
        f_dst1 = np.zeros(TOT1, np.int64)
        f_nrm1 = np.zeros(TOT1, np.float32)
        f_src1[pos1] = sl_d[order1]
        f_dst1[pos1] = dr_d[order1]
        f_nrm1[pos1] = nrm_d[order1]

        idx16 = np.ascontiguousarray(np.tile(f_src1.reshape(-1, 16).T, (8, 1)))
        ind1 = np.zeros((128, TOT1), BF16)
        posa = np.arange(TOT1)
        ind1[posa % 128, (posa // 128) * 128 + f_dst1] = f_nrm1.astype(BF16)

        indf = np.empty(128 * IND1_COLS, BF16)
        for s, subs in enumerate(sup_subs):
            base, spans = ind1_off[s]
            parts = [ind1[:, st:st + L] for st, L in spans]
            blk = np.concatenate(parts, axis=1)
            n = blk.shape[1]
            indf[128 * base:128 * (base + n)] = blk.reshape(-1)

        per_core.append({
            "idx16": idx16,
            "mi0": mi0,
            "indf": indf,
        })

    shared = {
        "w1t": np.ascontiguousarray(np.asarray(W1, np.float32).T.astype(BF16)),
        "w2t": np.ascontiguousarray(np.asarray(W2, np.float32).T.astype(BF16)),
        "wft": np.ascontiguousarray(np.asarray(Wf, np.float32).T.astype(BF16)),
        "b1col": np.asarray(b1, np.float32).reshape(128, 1).copy(),
        "b2col": np.asarray(b2, np.float32).reshape(128, 1).copy(),
        "bfbc": np.broadcast_to(np.asarray(bf, np.float32), (128, ENC)).copy(),
        "identb": np.eye(128, dtype=np.float32).astype(BF16),
    }
    meta = {
        "nb0": nb0, "off0": off0, "TOT0": TOT0,
        "nb1": nb1.reshape(NCHUNK, NSUB), "off1": off1, "TOT1": TOT1,
        "sup_subs": sup_subs, "mi0_off": mi0_off, "MI0_COLS": MI0_COLS,
        "ind1_off": ind1_off, "IND1_COLS": IND1_COLS,
    }
    return shared, per_core, meta


def _build(meta):
    """Build the SPMD bass program (identical for all 8 cores)."""
    nc = bacc.Bacc("TRN2", target_bir_lowering=False, debug=False,
                   num_devices=NCORES, num_swdge_queues=4)
    f32 = mybir.dt.float32
    bf16 = mybir.dt.bfloat16

    nb0, off0 = meta["nb0"], meta["off0"]
    nb1, off1, TOT1 = meta["nb1"], meta["off1"], meta["TOT1"]
    sup_subs, mi0_off, MI0_COLS = meta["sup_subs"], meta["mi0_off"], meta["MI0_COLS"]
    ind1_off, IND1_COLS = meta["ind1_off"], meta["IND1_COLS"]

    idx16_t = nc.dram_tensor("idx16", [128, TOT1 // 16], mybir.dt.int16, kind="ExternalInput")
    mi0_t = nc.dram_tensor("mi0", [128 * MI0_COLS], bf16, kind="ExternalInput")
    indf_t = nc.dram_tensor("indf", [128 * IND1_COLS], bf16, kind="ExternalInput")
    w1t_t = nc.dram_tensor("w1t", [F, HID], bf16, kind="ExternalInput")
    w2t_t = nc.dram_tensor("w2t", [HID, HID], bf16, kind="ExternalInput")
    wft_t = nc.dram_tensor("wft", [HID, ENC], bf16, kind="ExternalInput")
    b1col_t = nc.dram_tensor("b1col", [128, 1], f32, kind="ExternalInput")
    b2col_t = nc.dram_tensor("b2col", [128, 1], f32, kind="ExternalInput")
    bfbc_t = nc.dram_tensor("bfbc", [128, ENC], f32, kind="ExternalInput")
    identb_t = nc.dram_tensor("identb", [128, 128], bf16, kind="ExternalInput")
    out_t = nc.dram_tensor("out", [SHARD_PAD, ENC], f32, kind="ExternalOutput")

    blocks1 = [[(c, k) for c in range(NCHUNK) for k in range(int(nb1[c][t]))]
               for t in range(NSUB)]

    with tile.TileContext(nc) as tc:
        with tc.tile_pool(name="const", bufs=1) as cst, \
             tc.tile_pool(name="edata", bufs=1) as edata, \
             tc.tile_pool(name="mip", bufs=2) as mip, \
             tc.tile_pool(name="msgp", bufs=2) as msgp, \
             tc.tile_pool(name="indp", bufs=2) as indp, \
             tc.tile_pool(name="accp", bufs=3, space="PSUM") as accp, \
             tc.tile_pool(name="epsp", bufs=3, space="PSUM") as epsp, \
             tc.tile_pool(name="tpsp", bufs=2, space="PSUM") as tpsp, \
             tc.tile_pool(name="work", bufs=3) as work, \
             tc.tile_pool(name="dram", bufs=1, space="DRAM") as dram:

            # ---- persistent SBUF data ----
            idx_sb = edata.tile([128, TOT1 // 16], mybir.dt.int16)
            nc.sync.dma_start(idx_sb[:], idx16_t[:])

            w1t_sb = cst.tile([F, HID], bf16)
            w2t_sb = cst.tile([HID, HID], bf16)
            wft_sb = cst.tile([HID, ENC], bf16)
            b1col_sb = cst.tile([128, 1], f32)
            b2col_sb = cst.tile([128, 1], f32)
            bfbc_sb = cst.tile([128, ENC], f32)
            ident_sb = cst.tile([128, 128], bf16)
            for sb_, t_ in ((w1t_sb, w1t_t), (w2t_sb, w2t_t), (wft_sb, wft_t),
                            (b1col_sb, b1col_t), (b2col_sb, b2col_t),
                            (bfbc_sb, bfbc_t), (ident_sb, identb_t)):
                nc.sync.dma_start(sb_[:], t_[:])

            r1sh = dram.tile([SHARD_PAD, HID], bf16)
            r1full = dram.tile([XROWS, HID], bf16, addr_space="Shared")

            # ================= layer 0 =================
            def l0_load(s):
                base, L = mi0_off[s]
                mi = mip.tile([128, 2 * L], bf16, tag="mi")
                ap = mi0_t[128 * base:128 * (base + 2 * L)].rearrange(
                    "(p f) -> p f", p=128)
                nc.sync.dma_start(mi[:], ap)
                return mi, L

            staged = l0_load(0)
            for s in range(NSUP):
                subs = sup_subs[s]
                mi, L = staged
                if s + 1 < NSUP:
                    staged = l0_load(s + 1)
                st_sup = int(off0[subs[0]])
                for t in subs:
                    acc = accp.tile([128, 128], f32, tag="acc")
                    nbk = int(nb0[t])
                    o = int(off0[t]) - st_sup
                    for k in range(nbk):
                        nc.tensor.matmul(
                            acc[:],
                            lhsT=mi[:, o + k * 128:o + (k + 1) * 128],
                            rhs=mi[:, L + o + k * 128:L + o + (k + 1) * 128],
                            start=(k == 0), stop=(k == nbk - 1),
                        )
                    sum_sb = work.tile([128, 128], bf16, tag="sum")
                    nc.vector.tensor_copy(out=sum_sb[:], in_=acc[:])
                    z1 = epsp.tile([128, 128], f32, tag="eps")
                    nc.tensor.matmul(z1[:], lhsT=w1t_sb[:], rhs=sum_sb[:],
                                     start=True, stop=True)
                    r1t = work.tile([128, 128], bf16, tag="r1t")
                    nc.scalar.activation(
                        r1t[:], z1[:], mybir.ActivationFunctionType.Relu,
                        bias=b1col_sb[:, 0:1])
                    rp = tpsp.tile([128, 128], bf16, tag="tp")
                    nc.tensor.transpose(rp[:], r1t[:], ident_sb[:])
                    r1 = work.tile([128, HID], bf16, tag="r1")
                    nc.scalar.activation(
                        r1[:], rp[:], mybir.ActivationFunctionType.Copy)
                    nc.sync.dma_start(r1sh[t * 128:(t + 1) * 128, :], r1[:])

            # ================= collective =================
            nc.gpsimd.collective_compute(
                "AllGather",
                mybir.AluOpType.bypass,
                replica_groups=[list(range(NCORES))],
                ins=[r1sh[:].opt()],
                outs=[r1full[:].opt()],
            )

            # ================= layer 1 =================
            def l1_load(s):
                subs = sup_subs[s]
                base, spans = ind1_off[s]
                Lsum = sum(L for _, L in spans)
                ind = indp.tile([128, Lsum], bf16, tag="ind")
                ap = indf_t[128 * base:128 * (base + Lsum)].rearrange(
                    "(p f) -> p f", p=128)
                nc.scalar.dma_start(ind[:], ap)
                msgs = {}
                ind_off = {}
                io = 0
                for c in range(NCHUNK):
                    st, L = spans[c]
                    ind_off[c] = io
                    io += L
                    if L == 0:
                        continue
                    msg = msgp.tile([128, L], bf16, tag=f"msg{c}")
                    msgs[c] = msg
                    # split in halves: SDMA drain of half A overlaps Q7
                    # emission of half B
                    h = (L // 256) * 128
                    for a, b in ((0, h), (h, L)):
                        if b - a == 0:
                            continue
                        nc.gpsimd.dma_gather(
                            msg[:, a:b].rearrange("p (b f) -> p b f", f=128),
                            r1full[c * CHUNK:(c + 1) * CHUNK, :],
                            idx_sb[:, (st + a) // 16:(st + b) // 16],
                            b - a, b - a, 128, elem_step=F,
                            single_packet=False,
                            queue_num=c,
                        )
                return subs, msgs, ind, ind_off, {c: spans[c][0] for c in range(NCHUNK)}

            staged = l1_load(0)
            for s in range(NSUP):
                subs, msgs, ind, ind_off, starts = staged
                if s + 1 < NSUP:
                    staged = l1_load(s + 1)
                for t in subs:
                    acc = accp.tile([128, 128], f32, tag="acc")
                    for c in range(NCHUNK):
                        nbk = int(nb1[c][t])
                        if nbk == 0:
                            continue
                        mo = int(off1[c * NSUB + t]) - starts[c]
                        io = ind_off[c] + mo
                        for k in range(nbk):
                            nc.tensor.matmul(
                                acc[:],
                                lhsT=msgs[c][:, mo + k * 128:mo + (k + 1) * 128],
                                rhs=ind[:, io + k * 128:io + (k + 1) * 128],
                                start=(blocks1[t][0] == (c, k)),
                                stop=(blocks1[t][-1] == (c, k)),
                            )
                    sum_sb = work.tile([128, 128], bf16, tag="sum")
                    nc.vector.tensor_copy(out=sum_sb[:], in_=acc[:])
                    z2 = epsp.tile([128, 128], f32, tag="eps")
                    nc.tensor.matmul(z2[:], lhsT=w2t_sb[:], rhs=sum_sb[:],
                                     start=True, stop=True)
                    r2t = work.tile([128, 128], bf16, tag="r1t")
                    nc.scalar.activation(
                        r2t[:], z2[:], mybir.ActivationFunctionType.Relu,
                        bias=b2col_sb[:, 0:1])
                    fp = tpsp.tile([128, ENC], f32, tag="tp")
                    nc.tensor.matmul(fp[:], lhsT=r2t[:], rhs=wft_sb[:],
                                     start=True, stop=True)
                    fz = work.tile([128, ENC], f32, tag="fz")
                    nc.vector.tensor_tensor(out=fz[:], in0=fp[:],
                                            in1=bfbc_sb[:],
                                            op=mybir.AluOpType.add)
                    nc.sync.dma_start(out_t[t * 128:(t + 1) * 128, :], fz[:])

    nc.compile()
    return nc


def kernel(**inputs):
    shared, per_core, meta = _preprocess(
        inputs["x"], inputs["edge_index"], inputs["edge_weight"],
        inputs["W1"], inputs["b1"], inputs["W2"], inputs["b2"],
        inputs["Wf"], inputs["bf"])

    key = (meta["TOT0"], meta["TOT1"], meta["nb0"].tobytes(),
           meta["nb1"].tobytes())
    if key not in _cache:
        _cache[key] = _build(meta)
    nc = _cache[key]

    in_maps = []
    for d in range(NCORES):
        m = dict(shared)
        m.update(per_core[d])
        in_maps.append(m)

    res = bass_utils.run_bass_kernel_spmd(nc, in_maps, core_ids=list(range(NCORES)))
    out = np.concatenate(
        [res.results[d]["out"][:SHARD] for d in range(NCORES)], axis=0)
    return out.astype(np.float32)
